# revision 5
# baseline (speedup 1.0000x reference)
"""Distributed Trainium2 kernel for AlternateWeaveGather (segment_reduce).

Reference computation:
    h = x @ W.T + b                      # [N, 512] linear
    out = segment_mean(h, batch, 256)    # [256, 512]

Since the linear layer commutes with the segment sum:
    out[s] = (segsum_x[s] @ W.T) / max(c[s], 1) + b * (c[s] > 0)

so each core only needs to segment-reduce its row shard of x (one-hot
matmul on the TensorEngine), ReduceScatter the [256, 513] (sums|counts)
across the 8 cores, and run the tiny 512x512 linear on its 32 owned
segments. This turns a 68-GFLOP matmul problem into an HBM-bound
streaming reduction.

Sharding: data-parallel over rows. x/batch split along dim 0 across 8
cores; W/b replicated; ReduceScatter combines local sums+counts before
the divide; host concatenates the 8x[32, 512] outputs.
"""

import numpy as np

import concourse.bacc as bacc
import concourse.bass as bass
import concourse.mybir as mybir
import concourse.tile as tile
from concourse.bass_utils import run_bass_kernel_spmd

N_CORES = 8
N_ROWS = 131072
D = 512
N_SEG = 256
SEG_PER_CORE = N_SEG // N_CORES

F32 = mybir.dt.float32
F32R = mybir.dt.float32r


def build_nc(rows_per_core=N_ROWS // N_CORES):
    assert rows_per_core % 128 == 0
    n_tiles = rows_per_core // 128

    nc = bacc.Bacc("TRN2", target_bir_lowering=False, debug=False,
                   num_devices=N_CORES)
    x = nc.dram_tensor("x", [rows_per_core, D], F32, kind="ExternalInput")
    batchp = nc.dram_tensor("batchp", [128, n_tiles], F32, kind="ExternalInput")
    wt = nc.dram_tensor("wt", [D, D], F32, kind="ExternalInput")
    bb = nc.dram_tensor("bb", [SEG_PER_CORE, D], F32, kind="ExternalInput")
    out = nc.dram_tensor("out", [SEG_PER_CORE, D], F32, kind="ExternalOutput")

    iota_c = nc.inline_tensor(
        np.tile(np.arange(N_SEG, dtype=np.float32), (128, 1)), name="iota_c")
    ident_c = nc.inline_tensor(np.eye(128, dtype=np.float32), name="ident_c")
    ones_c = nc.inline_tensor(np.ones((128, 2), dtype=np.float32), name="ones_c")
    zeros_c = nc.inline_tensor(np.zeros((128, 512), dtype=np.float32), name="zeros_c")

    x_r = x.ap().rearrange("(t p) d -> t p d", p=128)
    RG = [list(range(N_CORES))]

    with tile.TileContext(nc) as tc:
        with tc.tile_pool(name="const", bufs=1) as const, \
             tc.tile_pool(name="dram", bufs=1, space="DRAM") as dram:
            iota_sb = const.tile([128, N_SEG], F32, name="iota_sb")
            ident_sb = const.tile([128, 128], F32, name="ident_sb")
            ones_sb = const.tile([128, 2], F32R, name="ones_sb")
            batch_sb = const.tile([128, n_tiles], F32, name="batch_sb")
            wt_sb = const.tile([128, 4 * D], F32R, name="wt_sb")
            b_sb = const.tile([SEG_PER_CORE, D], F32, name="b_sb")
            nc.sync.dma_start(out=iota_sb[:, :], in_=iota_c[:, :])
            nc.sync.dma_start(out=ident_sb[:, :], in_=ident_c[:, :])
            nc.sync.dma_start(out=batch_sb[:, :], in_=batchp[:, :])
            for i in range(4):
                nc.sync.dma_start(out=wt_sb[:, i * D:(i + 1) * D],
                                  in_=wt[i * 128:(i + 1) * 128, :].bitcast(F32R))
            nc.sync.dma_start(out=b_sb[:, :], in_=bb[:, :])
            nc.sync.dma_start(out=ones_sb[:, :], in_=ones_c[:, :].bitcast(F32R))

            rs_in = dram.tile([N_SEG, D + 1], F32, name="rs_in")
            rs_out = dram.tile([SEG_PER_CORE, D + 1], F32, name="rs_out")

            with tc.tile_pool(name="xin", bufs=4) as xp, \
                 tc.tile_pool(name="ohp", bufs=4) as ohp, \
                 tc.tile_pool(name="psum_acc", bufs=1, space="PSUM") as pacc:
                ps0 = pacc.tile([128, D], F32, name="ps0")
                ps1 = pacc.tile([128, D], F32, name="ps1")
                pc0 = pacc.tile([128, 2], F32, name="pc0")
                pc1 = pacc.tile([128, 2], F32, name="pc1")
                for t in range(n_tiles):
                    xt = xp.tile([128, D], F32R, name="xt")
                    nc.sync.dma_start(out=xt[:, :], in_=x_r[t].bitcast(F32R))
                    oh = ohp.tile([128, N_SEG], F32R, name="oh")
                    nc.vector.tensor_scalar(
                        oh[:, :], iota_sb[:, :], batch_sb[:, t:t + 1], None,
                        mybir.AluOpType.is_equal)
                    st, sp = (t == 0), (t == n_tiles - 1)
                    ohr = oh[:, :]
                    xtr = xt[:, :]
                    onesr = ones_sb[:, :]
                    nc.tensor.matmul(ps0[:, :], ohr[:, 0:128], xtr,
                                     start=st, stop=sp)
                    nc.tensor.matmul(pc0[:, :], ohr[:, 0:128], onesr,
                                     start=st, stop=sp)
                    nc.tensor.matmul(ps1[:, :], ohr[:, 128:256], xtr,
                                     start=st, stop=sp)
                    nc.tensor.matmul(pc1[:, :], ohr[:, 128:256], onesr,
                                     start=st, stop=sp)

                with tc.tile_pool(name="half", bufs=1) as halfp:
                    sb0 = halfp.tile([128, D + 1], F32, name="sb0")
                    sb1 = halfp.tile([128, D + 1], F32, name="sb1")
                    nc.vector.tensor_copy(sb0[:, 0:D], ps0[:, :])
                    nc.scalar.copy(sb1[:, 0:D], ps1[:, :])
                    nc.vector.tensor_copy(sb0[:, D:D + 1], pc0[:, 0:1])
                    nc.scalar.copy(sb1[:, D:D + 1], pc1[:, 0:1])
                    nc.sync.dma_start(out=rs_in[0:128, :], in_=sb0[:, :])
                    nc.sync.dma_start(out=rs_in[128:256, :], in_=sb1[:, :])

            nc.gpsimd.collective_compute(
                "ReduceScatter", mybir.AluOpType.add, replica_groups=RG,
                ins=[rs_in.opt()], outs=[rs_out.opt()])

            with tc.tile_pool(name="epi", bufs=1) as epi, \
                 tc.tile_pool(name="psum_epi", bufs=2, space="PSUM") as pepi:
                fin = epi.tile([SEG_PER_CORE, D + 1], F32, name="fin")
                nc.sync.dma_start(out=fin[:, :], in_=rs_out[:, :])
                cnt = fin[:, D:D + 1]
                cm = epi.tile([SEG_PER_CORE, 1], F32, name="cm")
                inv = epi.tile([SEG_PER_CORE, 1], F32, name="inv")
                ind = epi.tile([SEG_PER_CORE, 1], F32, name="ind")
                sc = epi.tile([SEG_PER_CORE, D], F32, name="sc")
                nc.vector.tensor_scalar_max(cm[:, :], cnt, 1.0)
                nc.vector.reciprocal(inv[:, :], cm[:, :])
                nc.vector.tensor_scalar_min(ind[:, :], cnt, 1.0)
                # sc = sums / max(c,1)
                nc.vector.tensor_scalar(sc[:, :], fin[:, 0:D], inv[:, 0:1],
                                        None, mybir.AluOpType.mult)

                # lhsT for the final matmul: transpose sc into 4 K-chunks,
                # each padded to M=128 (fp32r requires all column groups);
                # rows 32..127 of the result are never read.
                lhsT = epi.tile([128, 4 * 128], F32R, name="lhsT")
                nc.sync.dma_start(out=lhsT[:, :], in_=zeros_c[:, :].bitcast(F32R))
                for i in range(4):
                    pt = pepi.tile([128, SEG_PER_CORE], F32, name="pt",
                                   tag="pt")
                    nc.tensor.transpose(pt[:, :],
                                        sc[:, i * 128:(i + 1) * 128],
                                        ident_sb[0:SEG_PER_CORE, 0:SEG_PER_CORE])
                    nc.vector.tensor_copy(
                        lhsT[:, i * 128:i * 128 + SEG_PER_CORE], pt[:, :])

                po = pepi.tile([128, D], F32, name="po")
                for i in range(4):
                    nc.tensor.matmul(po[:, :],
                                     lhsT[:, i * 128:(i + 1) * 128],
                                     wt_sb[:, i * D:(i + 1) * D],
                                     start=(i == 0), stop=(i == 3))
                res = epi.tile([SEG_PER_CORE, D], F32, name="res")
                # res = b * min(c,1) + po  (bias gated by the empty-segment
                # indicator, fused on DVE)
                nc.vector.scalar_tensor_tensor(
                    res[:, :], b_sb[:, :], ind[:, 0:1], po[0:SEG_PER_CORE, :],
                    mybir.AluOpType.mult, mybir.AluOpType.add)
                nc.sync.dma_start(out=out[:, :], in_=res[:, :])
    nc.compile()
    return nc


def make_in_maps(x, W, b, batch, rows_per_core):
    x = np.asarray(x, dtype=np.float32)
    W = np.asarray(W, dtype=np.float32)
    b = np.asarray(b, dtype=np.float32)
    batch = np.asarray(batch)
    n_tiles = rows_per_core // 128
    wt = np.ascontiguousarray(W.T)
    bb = np.ascontiguousarray(np.tile(b.reshape(1, D), (SEG_PER_CORE, 1)))
    in_maps = []
    for j in range(N_CORES):
        lo = j * rows_per_core
        bp = batch[lo:lo + rows_per_core].astype(np.float32)
        bp = np.ascontiguousarray(bp.reshape(n_tiles, 128).T)
        in_maps.append({
            "x": np.ascontiguousarray(x[lo:lo + rows_per_core]),
            "batchp": bp,
            "wt": wt,
            "bb": bb,
        })
    return in_maps


_NC_CACHE = {}


def kernel(x, W, b, batch, num_segments, trace=False):
    assert int(num_segments) == N_SEG
    rows_per_core = N_ROWS // N_CORES
    if rows_per_core not in _NC_CACHE:
        _NC_CACHE[rows_per_core] = build_nc(rows_per_core)
    nc = _NC_CACHE[rows_per_core]
    in_maps = make_in_maps(x, W, b, batch, rows_per_core)
    res = run_bass_kernel_spmd(nc, in_maps, core_ids=list(range(N_CORES)),
                               trace=trace)
    full = np.concatenate([res.results[j]["out"] for j in range(N_CORES)],
                          axis=0)
    if trace:
        return full, res
    return full


# revision 8
# speedup vs baseline: 1.3338x; 1.3338x over previous
"""Distributed Trainium2 kernel for AlternateWeaveGather (segment_reduce).

Reference computation:
    h = x @ W.T + b                      # [N, 512] linear
    out = segment_mean(h, batch, 256)    # [256, 512]

Since the linear layer commutes with the segment sum:
    out[s] = (segsum_x[s] @ W.T) / max(c[s], 1) + b * (c[s] > 0)

so each core only needs to segment-reduce its row shard of x (one-hot
matmul on the TensorEngine), ReduceScatter the [256, 513] (sums|counts)
across the 8 cores, and run the tiny 512x512 linear on its 32 owned
segments. This turns a 68-GFLOP matmul problem into an HBM-bound
streaming reduction.

Because batch is sorted, each core's 16384 rows span only ~33 contiguous
segment ids, so the one-hot window is 128 wide (one matmul per 128 rows
instead of two 256-wide ones). The host passes batch ids relative to the
core's first segment; an indirect-scatter DMA realigns the local
[128, 513] window into global segment rows before the ReduceScatter.

Sharding: data-parallel over rows. x/batch split along dim 0 across 8
cores; W/b replicated; ReduceScatter combines local sums+counts before
the divide; host concatenates the 8x[32, 512] outputs.
"""

import numpy as np

import concourse.bacc as bacc
import concourse.bass as bass
import concourse.mybir as mybir
import concourse.tile as tile
from concourse.bass_utils import run_bass_kernel_spmd

N_CORES = 8
N_ROWS = 131072
D = 512
N_SEG = 256
SEG_PER_CORE = N_SEG // N_CORES
W_WIN = 128  # one-hot window width (per-core segment span is ~33)

F32 = mybir.dt.float32
F32R = mybir.dt.float32r
I32 = mybir.dt.int32


def build_nc(rows_per_core=N_ROWS // N_CORES):
    assert rows_per_core % 256 == 0
    n_sup = rows_per_core // 256  # supertiles of 256 rows (4KB DMA lines)

    nc = bacc.Bacc("TRN2", target_bir_lowering=False, debug=False,
                   num_devices=N_CORES)
    x = nc.dram_tensor("x", [rows_per_core, D], F32, kind="ExternalInput")
    # batchp[p, t]        = batch_rel[256t + 2p]     (plane a)
    # batchp[p, n_sup+t]  = batch_rel[256t + 2p + 1] (plane b)
    batchp = nc.dram_tensor("batchp", [128, 2 * n_sup], F32,
                            kind="ExternalInput")
    idx = nc.dram_tensor("idx", [128, 1], I32, kind="ExternalInput")
    wt = nc.dram_tensor("wt", [D, D], F32, kind="ExternalInput")
    bb = nc.dram_tensor("bb", [SEG_PER_CORE, D], F32, kind="ExternalInput")
    out = nc.dram_tensor("out", [SEG_PER_CORE, D], F32, kind="ExternalOutput")

    iota_c = nc.inline_tensor(
        np.tile(np.arange(W_WIN, dtype=np.float32), (128, 1)), name="iota_c")
    ident_c = nc.inline_tensor(np.eye(128, dtype=np.float32), name="ident_c")
    ones_c = nc.inline_tensor(np.ones((128, 2), dtype=np.float32),
                              name="ones_c")
    zeros_c = nc.inline_tensor(np.zeros((129, D + 1), dtype=np.float32),
                               name="zeros_c")

    # [n_sup, 128, 2, 512]; per (t, p) the (2, 512) block is 4KB contiguous
    x_r = x.ap().rearrange("(t p two) d -> t p two d", p=128, two=2)
    RG = [list(range(N_CORES))]

    with tile.TileContext(nc) as tc:
        with tc.tile_pool(name="const", bufs=1) as const, \
             tc.tile_pool(name="dram", bufs=1, space="DRAM") as dram:
            iota_sb = const.tile([128, W_WIN], F32, name="iota_sb")
            batch_sb = const.tile([128, 2 * n_sup], F32, name="batch_sb")
            idx_sb = const.tile([128, 1], I32, name="idx_sb")
            ident_sb = const.tile([128, 128], F32, name="ident_sb")
            ones_sb = const.tile([128, 2], F32R, name="ones_sb")
            wt_sb = const.tile([128, 4 * D], F32R, name="wt_sb")
            b_sb = const.tile([SEG_PER_CORE, D], F32, name="b_sb")
            ohacc = const.tile([128, W_WIN], F32, name="ohacc")
            nc.sync.dma_start(out=iota_sb[:, :], in_=iota_c[:, :])
            nc.sync.dma_start(out=batch_sb[:, :], in_=batchp[:, :])
            nc.gpsimd.dma_start(out=idx_sb[:, :], in_=idx[:, :])
            nc.gpsimd.dma_start(out=ones_sb[:, :],
                                in_=ones_c[:, :].bitcast(F32R))
            nc.gpsimd.dma_start(out=ohacc[:, :],
                                in_=zeros_c[0:128, 0:W_WIN])

            rs_in = dram.tile([N_SEG + 1, D + 1], F32, name="rs_in")
            rs_out = dram.tile([SEG_PER_CORE, D + 1], F32, name="rs_out")
            # zero the scatter target (only 128 of 257 rows get data)
            nc.gpsimd.dma_start(out=rs_in[0:129, :], in_=zeros_c[:, :])
            nc.gpsimd.dma_start(out=rs_in[129:257, :],
                                in_=zeros_c[0:128, :])

            with tc.tile_pool(name="xin", bufs=6) as xp, \
                 tc.tile_pool(name="ohp", bufs=6) as ohp, \
                 tc.tile_pool(name="psum_acc", bufs=1, space="PSUM") as pacc:
                ps = pacc.tile([128, D], F32, name="ps")
                for t in range(n_sup):
                    xt = xp.tile([128, 2, D], F32R, name="xt")
                    nc.sync.dma_start(out=xt[:, :, :],
                                      in_=x_r[t].bitcast(F32R))
                    st, sp0 = (t == 0), (t == n_sup - 1)
                    for half in range(2):
                        oh = ohp.tile([128, W_WIN], F32R, name="oh")
                        nc.vector.tensor_scalar(
                            oh[:, :], iota_sb[:, :],
                            batch_sb[:, half * n_sup + t:half * n_sup + t + 1],
                            None, mybir.AluOpType.is_equal)
                        nc.tensor.matmul(ps[:, :], oh[:, :], xt[:, half, :],
                                         start=(st and half == 0),
                                         stop=(sp0 and half == 1))
                        nc.gpsimd.tensor_tensor(
                            ohacc[:, :], ohacc[:, :], oh[:, :].bitcast(F32),
                            mybir.AluOpType.add)

                with tc.tile_pool(name="half", bufs=1) as halfp, \
                     tc.tile_pool(name="psum_cnt", bufs=1,
                                  space="PSUM") as pcnt:
                    # counts: column sums of the accumulated one-hots
                    pc = pcnt.tile([128, 2], F32, name="pc")
                    nc.tensor.matmul(pc[:, :], ohacc[:, :],
                                     ones_sb[:, :].bitcast(F32),
                                     start=True, stop=True)
                    sbw = halfp.tile([128, D + 1], F32, name="sbw")
                    nc.vector.tensor_copy(sbw[:, 0:D], ps[:, :])
                    nc.scalar.copy(sbw[:, D:D + 1], pc[:, 0:1])
                    # place the local window at its global segment rows
                    # (row p -> rs_in[idx[p]]; idx clamps overflow to the
                    # dummy row 256)
                    nc.gpsimd.indirect_dma_start(
                        out=rs_in[:, :], out_offset=bass.IndirectOffsetOnAxis(
                            ap=idx_sb[:, 0:1], axis=0),
                        in_=sbw[:, :], in_offset=None)

            nc.gpsimd.collective_compute(
                "ReduceScatter", mybir.AluOpType.add, replica_groups=RG,
                ins=[rs_in[0:N_SEG, :].opt()], outs=[rs_out.opt()])

            # epilogue inputs (only needed after the collective)
            for i in range(4):
                nc.scalar.dma_start(out=wt_sb[:, i * D:(i + 1) * D],
                                    in_=wt[i * 128:(i + 1) * 128, :].bitcast(F32R))
            nc.scalar.dma_start(out=b_sb[:, :], in_=bb[:, :])
            nc.scalar.dma_start(out=ident_sb[:, :], in_=ident_c[:, :])

            with tc.tile_pool(name="epi", bufs=1) as epi, \
                 tc.tile_pool(name="psum_epi", bufs=2, space="PSUM") as pepi:
                fin = epi.tile([SEG_PER_CORE, D + 1], F32, name="fin")
                nc.sync.dma_start(out=fin[:, :], in_=rs_out[:, :])
                cnt = fin[:, D:D + 1]
                cm = epi.tile([SEG_PER_CORE, 1], F32, name="cm")
                inv = epi.tile([SEG_PER_CORE, 1], F32, name="inv")
                ind = epi.tile([SEG_PER_CORE, 1], F32, name="ind")
                sc = epi.tile([SEG_PER_CORE, D], F32, name="sc")
                nc.vector.tensor_scalar_max(cm[:, :], cnt, 1.0)
                nc.vector.reciprocal(inv[:, :], cm[:, :])
                nc.vector.tensor_scalar_min(ind[:, :], cnt, 1.0)
                # sc = sums / max(c,1)
                nc.vector.tensor_scalar(sc[:, :], fin[:, 0:D], inv[:, 0:1],
                                        None, mybir.AluOpType.mult)

                # lhsT for the final matmul: transpose sc into 4 K-chunks,
                # each padded to M=128 (fp32r requires all column groups);
                # rows 32..127 of the result are never read.
                lhsT = epi.tile([128, 4 * 128], F32R, name="lhsT")
                nc.sync.dma_start(out=lhsT[:, :],
                                  in_=zeros_c[0:128, 0:D].bitcast(F32R))
                for i in range(4):
                    pt = pepi.tile([128, SEG_PER_CORE], F32, name="pt",
                                   tag="pt")
                    nc.tensor.transpose(pt[:, :],
                                        sc[:, i * 128:(i + 1) * 128],
                                        ident_sb[0:SEG_PER_CORE, 0:SEG_PER_CORE])
                    nc.vector.tensor_copy(
                        lhsT[:, i * 128:i * 128 + SEG_PER_CORE], pt[:, :])

                po = pepi.tile([128, D], F32, name="po")
                for i in range(4):
                    nc.tensor.matmul(po[:, :],
                                     lhsT[:, i * 128:(i + 1) * 128],
                                     wt_sb[:, i * D:(i + 1) * D],
                                     start=(i == 0), stop=(i == 3))
                res = epi.tile([SEG_PER_CORE, D], F32, name="res")
                # res = b * min(c,1) + po  (bias gated by the empty-segment
                # indicator, fused on DVE)
                nc.vector.scalar_tensor_tensor(
                    res[:, :], b_sb[:, :], ind[:, 0:1], po[0:SEG_PER_CORE, :],
                    mybir.AluOpType.mult, mybir.AluOpType.add)
                nc.sync.dma_start(out=out[:, :], in_=res[:, :])
    nc.compile()
    return nc


def make_in_maps(x, W, b, batch, rows_per_core):
    x = np.asarray(x, dtype=np.float32)
    W = np.asarray(W, dtype=np.float32)
    b = np.asarray(b, dtype=np.float32)
    batch = np.asarray(batch)
    n_sup = rows_per_core // 256
    wt = np.ascontiguousarray(W.T)
    bb = np.ascontiguousarray(np.tile(b.reshape(1, D), (SEG_PER_CORE, 1)))
    in_maps = []
    for j in range(N_CORES):
        lo = j * rows_per_core
        bs = batch[lo:lo + rows_per_core].astype(np.int64)
        base = int(bs[0])
        rel = (bs - base).astype(np.float32)
        assert rel.max() < W_WIN, (
            f"core {j}: segment span {int(rel.max()) + 1} exceeds window")
        planes = rel.reshape(n_sup, 128, 2)
        bp = np.concatenate([planes[:, :, 0].T, planes[:, :, 1].T], axis=1)
        rowidx = np.minimum(base + np.arange(128), N_SEG).astype(np.int32)
        in_maps.append({
            "x": np.ascontiguousarray(x[lo:lo + rows_per_core]),
            "batchp": np.ascontiguousarray(bp),
            "idx": np.ascontiguousarray(rowidx.reshape(128, 1)),
            "wt": wt,
            "bb": bb,
        })
    return in_maps


_NC_CACHE = {}


def kernel(x, W, b, batch, num_segments, trace=False):
    assert int(num_segments) == N_SEG
    rows_per_core = N_ROWS // N_CORES
    if rows_per_core not in _NC_CACHE:
        _NC_CACHE[rows_per_core] = build_nc(rows_per_core)
    nc = _NC_CACHE[rows_per_core]
    in_maps = make_in_maps(x, W, b, batch, rows_per_core)
    res = run_bass_kernel_spmd(nc, in_maps, core_ids=list(range(N_CORES)),
                               trace=trace)
    full = np.concatenate([res.results[j]["out"] for j in range(N_CORES)],
                          axis=0)
    if trace:
        return full, res
    return full
